# revision 1
# baseline (speedup 1.0000x reference)
"""Trainium2 Bass kernel for Llama-like attention (16 heads, tanh softcap, RoPE).

Sharding: tensor-parallel over heads. Each of the 8 cores computes 2 heads:
  - q/k/v projections with column-sliced weights (x^T resident in SBUF,
    weight-slice streamed): q/k/v in natural [s, d] layout, f32 PSUM.
  - RoPE applied in natural layout. Weight columns of wq/wk are pre-permuted
    on the host to de-interleave even/odd rotary pairs, so rope works on
    contiguous 64-wide slices (the d-permutation cancels inside q.k).
  - attention with scores computed transposed ([kj, qi]) so the softmaxed
    probabilities feed the PV matmul directly as the moving operand.
    tanh softcap bounds scores, so softmax needs no row-max pass:
    p = exp(50*tanh(qk/(50*sqrt(hd)))), l = ones-row matmul, o = p@v / l.
  - per-head AllGather of o^T across cores, then each core contracts the
    full gathered o^T with its 256-column slice of wo and returns the
    transposed output slice; the host reassembles and transposes.
"""

import os
import sys

for _p in ("/root/.axon_site/_ro/trn_rl_repo", "/opt/trn_rl_repo"):
    if os.path.isdir(_p) and _p not in sys.path:
        sys.path.append(_p)

import numpy as np
import ml_dtypes
from contextlib import ExitStack

import concourse.bass as bass
import concourse.bacc as bacc
import concourse.mybir as mybir
import concourse.tile as tile
from concourse.bass_utils import run_bass_kernel_spmd
from concourse.masks import make_identity

BF16 = mybir.dt.bfloat16
F32 = mybir.dt.float32
NPBF16 = ml_dtypes.bfloat16

N_CORES = 8
S = 2048          # sequence length
DM = 2048         # model dim
H = 16            # heads
HD = 128          # head dim
HPC = H // N_CORES  # heads per core = 2
CW = HPC * HD     # per-core projection width = 256
P = 128
QT = 512          # query tile (free dim of attention matmuls)
NQT = S // QT     # 4 query tiles per head
NSC = S // P      # 16 sequence chunks
NKC = DM // P     # 16 contraction chunks
SOFTCAP = 50.0
C1 = 1.0 / (SOFTCAP * np.sqrt(HD))

Tanh = mybir.ActivationFunctionType.Tanh
Exp = mybir.ActivationFunctionType.Exp


def build_nc(reps=1, single=False):
    nc = bacc.Bacc("TRN2", target_bir_lowering=False, num_devices=N_CORES)

    xT_d = nc.dram_tensor("xT", [DM, S], BF16, kind="ExternalInput")
    w_d = nc.dram_tensor("w_all", [DM, 3 * CW], BF16, kind="ExternalInput")
    wo_d = nc.dram_tensor("wo_c", [DM, CW], BF16, kind="ExternalInput")
    cos_d = nc.dram_tensor("cos_b", [S, HD // 2], BF16, kind="ExternalInput")
    sin_d = nc.dram_tensor("sin_b", [S, HD // 2], BF16, kind="ExternalInput")
    mask_d = nc.dram_tensor("mask", [P, 4 * QT], BF16, kind="ExternalInput")
    out_d = nc.dram_tensor("outT", [CW, S], F32, kind="ExternalOutput")

    # collective bounce buffers (one per local head)
    ob = [nc.dram_tensor(f"ob{j}", [P, S], BF16) for j in range(HPC)]
    og = [
        nc.dram_tensor(f"og{j}", [N_CORES * P, S], BF16, addr_space="Shared")
        for j in range(HPC)
    ]

    with tile.TileContext(nc) as tc:
        for _rep in range(reps):
            _emit_body(nc, tc, xT_d, w_d, wo_d, cos_d, sin_d, mask_d, out_d,
                       ob, og, single)
    nc.compile()
    return nc


def _emit_body(nc, tc, xT_d, w_d, wo_d, cos_d, sin_d, mask_d, out_d, ob, og,
               single):
        with ExitStack() as ctx:
            # ---------- persistent SBUF ----------
            persist = ctx.enter_context(tc.tile_pool(name="persist", bufs=1))
            qT = [persist.tile([P, S], BF16, name=f"qT{h}") for h in range(HPC)]
            kT = [persist.tile([P, S], BF16, name=f"kT{h}") for h in range(HPC)]
            v_sb = [persist.tile([P, S], BF16, name=f"v{h}") for h in range(HPC)]
            oT = [persist.tile([P, S], BF16, name=f"oT{h}") for h in range(HPC)]
            mask_sb = persist.tile([P, 4 * QT], BF16, name="mask")
            ident = persist.tile([P, P], BF16, name="ident")
            ones_bf = persist.tile([P, 1], BF16, name="ones")
            cos_sb = persist.tile([P, NSC, HD // 2], BF16, name="cos")
            sin_sb = persist.tile([P, NSC, HD // 2], BF16, name="sin")

            nc.sync.dma_start(out=mask_sb[:], in_=mask_d[:])
            make_identity(nc, ident[:])
            nc.vector.memset(ones_bf[:], 1.0)
            # cos/sin: [S, 32] viewed as [NSC, P, 32] -> [P, NSC, 32]
            cos_r = cos_d.rearrange("(n p) f -> n p f", p=P)
            sin_r = sin_d.rearrange("(n p) f -> n p f", p=P)
            for i in range(NSC):
                nc.sync.dma_start(out=cos_sb[:, i, :], in_=cos_r[i])
                nc.sync.dma_start(out=sin_sb[:, i, :], in_=sin_r[i])

            # ---------- phase A: qkv projections + rope + transpose ----------
            with ExitStack() as ctxA:
                xp = ctxA.enter_context(tc.tile_pool(name="xT", bufs=1))
                wp = ctxA.enter_context(tc.tile_pool(name="w", bufs=1))
                rp = ctxA.enter_context(tc.tile_pool(name="rope", bufs=3))
                tmp = ctxA.enter_context(tc.tile_pool(name="ropetmp", bufs=4))
                qkv_ps = ctxA.enter_context(
                    tc.tile_pool(name="qkv_ps", bufs=2, space="PSUM")
                )
                tp_ps = ctxA.enter_context(
                    tc.tile_pool(name="tp_ps", bufs=2, space="PSUM")
                )

                # x^T split into 4 column groups so the first s-chunk's
                # matmuls only wait on the first quarter of the load
                NXQ = int(os.environ.get('KQ_NXQ', '4'))
                XQW = S // NXQ
                xt = [[xp.tile([P, XQW], BF16, name=f"xt{k}_{q}")
                       for q in range(NXQ)] for k in range(NKC)]
                wt = [wp.tile([P, 3 * CW], BF16, name=f"wt{k}") for k in range(NKC)]
                for k in range(NKC):
                    nc.sync.dma_start(out=wt[k][:], in_=w_d[k * P:(k + 1) * P, :])
                for q in range(NXQ):
                    for k in range(NKC):
                        nc.sync.dma_start(
                            out=xt[k][q][:],
                            in_=xT_d[k * P:(k + 1) * P, q * XQW:(q + 1) * XQW])

                HW = HD // 2  # 64
                for sc in range(NSC):
                    ps = qkv_ps.tile([P, 3 * CW], F32, name="qkv")
                    for k in range(NKC):
                        cpg = NSC // NXQ
                        lhsT = xt[k][sc // cpg][:, (sc % cpg) * P:(sc % cpg + 1) * P]
                        nc.tensor.matmul(
                            ps[:, 0:512], lhsT, wt[k][:, 0:512],
                            start=(k == 0), stop=(k == NKC - 1),
                        )
                        nc.tensor.matmul(
                            ps[:, 512:768], lhsT, wt[k][:, 512:768],
                            start=(k == 0), stop=(k == NKC - 1),
                        )
                    q_sb = rp.tile([P, CW], BF16, name="q_sb")
                    k_sb = rp.tile([P, CW], BF16, name="k_sb")
                    nc.scalar.copy(q_sb[:], ps[:, 0:CW])
                    nc.scalar.copy(k_sb[:], ps[:, CW:2 * CW])
                    for h in range(HPC):
                        nc.vector.tensor_copy(
                            v_sb[h][:, sc * P:(sc + 1) * P],
                            ps[:, 2 * CW + h * HD:2 * CW + (h + 1) * HD],
                        )
                    c_ap = cos_sb[:, sc, :]
                    s_ap = sin_sb[:, sc, :]
                    for src, rotT in ((q_sb, qT), (k_sb, kT)):
                        rot = rp.tile([P, CW], BF16, name="rot")
                        for h in range(HPC):
                            x0 = src[:, h * HD:h * HD + HW]
                            x1 = src[:, h * HD + HW:(h + 1) * HD]
                            t1 = tmp.tile([P, HW], BF16, name="t1")
                            t2 = tmp.tile([P, HW], BF16, name="t2")
                            nc.vector.tensor_mul(t1[:], x0, c_ap)
                            nc.vector.tensor_mul(t2[:], x1, s_ap)
                            nc.vector.tensor_sub(
                                rot[:, h * HD:h * HD + HW], t1[:], t2[:])
                            t3 = tmp.tile([P, HW], BF16, name="t3")
                            t4 = tmp.tile([P, HW], BF16, name="t4")
                            nc.vector.tensor_mul(t3[:], x0, s_ap)
                            nc.vector.tensor_mul(t4[:], x1, c_ap)
                            nc.vector.tensor_add(
                                rot[:, h * HD + HW:(h + 1) * HD], t3[:], t4[:])
                        for h in range(HPC):
                            tp = tp_ps.tile([P, P], BF16, name="tp")
                            nc.tensor.transpose(
                                tp[:], rot[:, h * HD:(h + 1) * HD], ident[:])
                            nc.vector.tensor_copy(
                                rotT[h][:, sc * P:(sc + 1) * P], tp[:])

            # ---------- phase B: attention ----------
            wop = ctx.enter_context(tc.tile_pool(name="wo", bufs=1))
            og0p = ctx.enter_context(tc.tile_pool(name="og0", bufs=4 * N_CORES))
            wo_sb = [wop.tile([P, CW], BF16, name=f"wo{i}") for i in range(NKC)]
            for i in range(NKC):
                nc.sync.dma_start(out=wo_sb[i][:], in_=wo_d[i * P:(i + 1) * P, :])
            og0t = {}
            with ExitStack() as ctxB:
                s_ps = ctxB.enter_context(
                    tc.tile_pool(name="s_ps", bufs=2, space="PSUM"))
                o_ps = ctxB.enter_context(
                    tc.tile_pool(name="o_ps", bufs=2, space="PSUM"))
                l_ps = ctxB.enter_context(
                    tc.tile_pool(name="l_ps", bufs=2, space="PSUM"))
                thp = ctxB.enter_context(tc.tile_pool(name="tanh", bufs=3))
                pp = ctxB.enter_context(tc.tile_pool(name="pT", bufs=3))
                np_ = ctxB.enter_context(tc.tile_pool(name="norm", bufs=2))

                for h in range(HPC):
                    for t in range(NQT):
                        o_acc = o_ps.tile([P, QT], F32, name="o_acc")
                        l_acc = l_ps.tile([1, QT], F32, name="l_acc")
                        npair = 2 * t + 2
                        q_ap = qT[h][:, t * QT:(t + 1) * QT]

                        def emit_pv(pT, p, last):
                            for i in range(2):
                                kc = 2 * p + i
                                nc.tensor.matmul(
                                    o_acc[:],
                                    v_sb[h][:, kc * P:(kc + 1) * P],
                                    pT[:, i * QT:(i + 1) * QT],
                                    start=(kc == 0), stop=(last and i == 1),
                                )
                                nc.tensor.matmul(
                                    l_acc[:], ones_bf[:, 0:1],
                                    pT[:, i * QT:(i + 1) * QT],
                                    start=(kc == 0), stop=(last and i == 1),
                                )

                        prev = None
                        for p in range(npair):
                            # scores for chunk pair (2p, 2p+1), each [P, QT]
                            sp = s_ps.tile([P, 2 * QT], F32, name="sp")
                            for i in range(2):
                                kc = 2 * p + i
                                nc.tensor.matmul(
                                    sp[:, i * QT:(i + 1) * QT],
                                    kT[h][:, kc * P:(kc + 1) * P], q_ap,
                                    start=True, stop=True,
                                )
                            th = thp.tile([P, 2 * QT], F32, name="th")
                            nc.scalar.activation(th[:], sp[:], Tanh, scale=C1)
                            pT = pp.tile([P, 2 * QT], BF16, name="pTt")
                            nc.scalar.activation(pT[:], th[:], Exp, scale=SOFTCAP)
                            # masked pairs are the last two: p==2t (u=0,1) and
                            # p==2t+1 (u=2,3); mask_sb columns line up 1:1
                            u0 = 2 * (p - 2 * t)
                            if u0 >= 0:
                                nc.vector.tensor_mul(
                                    pT[:], pT[:],
                                    mask_sb[:, u0 * QT:(u0 + 2) * QT])
                            if prev is not None:
                                emit_pv(prev[0], prev[1], last=False)
                            prev = (pT, p)
                        emit_pv(prev[0], prev[1], last=True)
                        recip = np_.tile([1, QT], F32, name="recip")
                        nc.vector.reciprocal(recip[:], l_acc[:])
                        bcast = np_.tile([P, QT], F32, name="bcast")
                        nc.gpsimd.partition_broadcast(bcast[:], recip[:])
                        nc.vector.tensor_mul(
                            oT[h][:, t * QT:(t + 1) * QT], o_acc[:], bcast[:])
                    nc.sync.dma_start(out=ob[h][:], in_=oT[h][:])
                    if single:
                        # timeline-sim stand-in for the AllGather
                        nc.gpsimd.dma_start(out=og[h][0:P, :], in_=ob[h][:])
                    else:
                        nc.gpsimd.collective_compute(
                            "AllGather", mybir.AluOpType.bypass,
                            replica_groups=[list(range(N_CORES))],
                            ins=[ob[h][:]], outs=[og[h][:]],
                        )
                    if h == 0:
                        # stream gathered head-0 o^T into SBUF during
                        # head 1's (ACT-bound) attention
                        for n in range(NQT):
                            for k in range(N_CORES):
                                t_ = og0p.tile([P, QT], BF16, name="og0t")
                                nc.sync.dma_start(
                                    out=t_[:],
                                    in_=og[0][k * P:(k + 1) * P,
                                              n * QT:(n + 1) * QT],
                                )
                                og0t[n, k] = t_

            # ---------- phase C: output projection ----------
            # The og0 half of the contraction is emitted first so it runs on
            # PE while the second AllGather is still in flight; og1 closes
            # the accumulation groups.
            with ExitStack() as ctxC:
                og1p = ctxC.enter_context(tc.tile_pool(name="og1", bufs=4 * N_CORES))
                outp = ctxC.enter_context(tc.tile_pool(name="out", bufs=3))
                wo_ps = ctxC.enter_context(
                    tc.tile_pool(name="wo_ps", bufs=NQT * HPC, space="PSUM"))

                accs = {}
                for n in range(NQT):
                    for m in range(HPC):
                        acc = wo_ps.tile([P, QT], F32, name="acc")
                        accs[n, m] = acc
                        for k in range(N_CORES):
                            nc.tensor.matmul(
                                acc[:], wo_sb[k][:, m * P:(m + 1) * P],
                                og0t[n, k][:],
                                start=(k == 0), stop=False,
                            )
                og1t = {}
                for n in range(NQT):
                    for k in range(N_CORES):
                        t_ = og1p.tile([P, QT], BF16, name="og1t")
                        nc.sync.dma_start(
                            out=t_[:],
                            in_=og[1][k * P:(k + 1) * P, n * QT:(n + 1) * QT],
                        )
                        og1t[n, k] = t_
                for n in range(NQT):
                    for m in range(HPC):
                        acc = accs[n, m]
                        for k in range(N_CORES):
                            nc.tensor.matmul(
                                acc[:], wo_sb[N_CORES + k][:, m * P:(m + 1) * P],
                                og1t[n, k][:],
                                start=False, stop=(k == N_CORES - 1),
                            )
                        osb = outp.tile([P, QT], F32, name="osb")
                        nc.scalar.copy(osb[:], acc[:])
                        nc.sync.dma_start(
                            out=out_d[m * P:(m + 1) * P, n * QT:(n + 1) * QT],
                            in_=osb[:],
                        )


_NC_CACHE = None


def _get_nc():
    global _NC_CACHE
    if _NC_CACHE is None:
        _NC_CACHE = build_nc()
    return _NC_CACHE


def _rope_perm():
    """per-head column permutation de-interleaving rotary pairs"""
    perm = np.zeros(DM, np.int64)
    for h in range(H):
        base = h * HD
        perm[base:base + HD // 2] = base + np.arange(0, HD, 2)
        perm[base + HD // 2:base + HD] = base + np.arange(1, HD, 2)
    return perm


def make_in_maps(x, wq, wk, wv, wo, freqs_cos, freqs_sin):
    x = np.asarray(x, np.float32).reshape(S, DM)
    wq = np.asarray(wq, np.float32)
    wk = np.asarray(wk, np.float32)
    wv = np.asarray(wv, np.float32)
    wo = np.asarray(wo, np.float32)
    xT = np.ascontiguousarray(x.T).astype(NPBF16)
    perm = _rope_perm()
    wq_p = wq[:, perm]
    wk_p = wk[:, perm]
    cos_b = np.asarray(freqs_cos, np.float32).astype(NPBF16)
    sin_b = np.asarray(freqs_sin, np.float32).astype(NPBF16)
    # mask[i, u*QT + j] = 1 if i <= j - 128*u else 0  (keep kj <= qi)
    i_idx = np.arange(P)[:, None]
    j_idx = np.arange(QT)[None, :]
    mask = np.concatenate(
        [(i_idx <= j_idx - P * u) for u in range(4)], axis=1
    ).astype(NPBF16)
    # wo rows reordered to match AllGather row order: og[j] rows are
    # (core r, local head j) = global head 2r+j
    wo_r = np.concatenate(
        [
            np.concatenate(
                [wo[(HPC * r + j) * HD:(HPC * r + j + 1) * HD, :]
                 for r in range(N_CORES)], axis=0)
            for j in range(HPC)
        ],
        axis=0,
    )
    in_maps = []
    for c in range(N_CORES):
        cs = slice(c * CW, (c + 1) * CW)
        w_all = np.concatenate(
            [wq_p[:, cs], wk_p[:, cs], wv[:, cs]], axis=1).astype(NPBF16)
        wo_c = np.ascontiguousarray(wo_r[:, cs]).astype(NPBF16)
        in_maps.append({
            "xT": xT,
            "w_all": np.ascontiguousarray(w_all),
            "wo_c": wo_c,
            "cos_b": cos_b,
            "sin_b": sin_b,
            "mask": mask,
        })
    return in_maps


def assemble_output(results):
    outT = np.concatenate([r["outT"] for r in results], axis=0)  # [DM, S]
    return np.ascontiguousarray(outT.T).reshape(1, S, DM).astype(np.float32)


def kernel(x, wq, wk, wv, wo, freqs_cos, freqs_sin):
    nc = _get_nc()
    in_maps = make_in_maps(x, wq, wk, wv, wo, freqs_cos, freqs_sin)
    res = run_bass_kernel_spmd(nc, in_maps, core_ids=list(range(N_CORES)))
    return assemble_output(res.results)


if __name__ == "__main__":
    rng = np.random.default_rng(0)
    ins = {
        "x": rng.standard_normal((1, S, DM), np.float32),
        "wq": rng.standard_normal((DM, DM), np.float32) / np.sqrt(DM),
        "wk": rng.standard_normal((DM, DM), np.float32) / np.sqrt(DM),
        "wv": rng.standard_normal((DM, DM), np.float32) / np.sqrt(DM),
        "wo": rng.standard_normal((DM, DM), np.float32) / np.sqrt(DM),
        "freqs_cos": rng.standard_normal((S, HD // 2), np.float32),
        "freqs_sin": rng.standard_normal((S, HD // 2), np.float32),
    }
    out = kernel(**ins)
    print("out", out.shape, out.dtype, np.abs(out).mean())

